# revision 12
# baseline (speedup 1.0000x reference)
"""Trainium2 Bass kernel for nn_CustomLossNN_52664888984291.

Computes: CrossEntropyLoss(logits, targets) + 10.0 * sum(P - uniq_per_row)
for logits [4096, 32000] f32, targets [4096] int.

Strategy (data-parallel over 8 NeuronCores, batch dim):
  - Each core streams its [512, 32000] logits shard from HBM once and
    computes per-row sum(exp(x)) on-device: ScalarE Exp activation with
    accum_out does exp + free-axis sum in a single pass; DMA is the
    bottleneck (memory-bound roofline ~65.5 MB/core @ ~358 GB/s).
  - Host finishes the scalar: lse = log(sumexp), gathers the target
    logit per row (4096 elements, negligible), ce = mean(lse - x[i,t_i]),
    and adds the shape-derived unique-count penalty
    (targets.reshape(B,-1) is [B,1] -> uniq=1 -> C-1 repeated per row).

Raw Bass (not Tile): walrus limits dynamic DMA instructions to a single
attached sync-wait, so waits are emitted as standalone wait_ge on the
engine queues. Every DMA wait is exact-max (its own semaphore at full
count), so SDMA engine skew cannot alias a wait to an incomplete DMA.
"""

import sys
from contextlib import ExitStack

import numpy as np

if "/opt/trn_rl_repo" not in sys.path:
    sys.path.insert(0, "/opt/trn_rl_repo")

import concourse.bass as bass
import concourse.mybir as mybir
from concourse.bass_utils import run_bass_kernel_spmd

B, C = 4096, 32000
N_CORES = 8
ROWS_PER_CORE = B // N_CORES  # 512
P = 128  # SBUF partitions
COL_CHUNK = 8000
BUFS = 4
PENALTY = 10.0

_NC = None


def _build_nc(
    rows_per_core=ROWS_PER_CORE,
    ncols=C,
    col_chunk=COL_CHUNK,
    bufs=BUFS,
    repeat=1,
    queues=1,
    internal_src=False,
):
    """repeat > 1 re-runs the whole pipeline over the same input; used only
    for benchmarking (marginal wall time per extra repeat = HW kernel time).
    internal_src=True streams from an uninitialized internal DRAM tensor so
    benchmark calls skip the 524 MB host->device transfer."""
    row_tiles = rows_per_core // P
    n_chunks = ncols // col_chunk
    n_tiles = row_tiles * n_chunks
    g_tiles = n_tiles * repeat
    g_rtiles = row_tiles * repeat
    f32 = mybir.dt.float32

    nc = bass.Bass()
    if internal_src:
        x = nc.dram_tensor("x", [rows_per_core, ncols], f32)
    else:
        x = nc.dram_tensor("x", [rows_per_core, ncols], f32, kind="ExternalInput")
    out = nc.dram_tensor("out", [P, g_rtiles], f32, kind="ExternalOutput")

    with ExitStack() as ctx:
        inp = [
            ctx.enter_context(nc.sbuf_tensor(f"inp{i}", [P, col_chunk], f32))
            for i in range(bufs)
        ]
        stats = ctx.enter_context(nc.sbuf_tensor("stats", [P, g_tiles], f32))
        sumexp = ctx.enter_context(nc.sbuf_tensor("sumexp", [P, g_rtiles], f32))

        load_sems = [
            ctx.enter_context(nc.semaphore(f"load{k}")) for k in range(n_tiles)
        ]
        act_sem = ctx.enter_context(nc.semaphore("act_sem"))
        dve_sem = ctx.enter_context(nc.semaphore("dve_sem"))
        out_sem = ctx.enter_context(nc.semaphore("out_sem"))
        block = ctx.enter_context(nc.Block())

        def load_prog(eng, q):
            # queue q issues loads g where g % queues == q; overlapping the
            # per-DMA SEQ/DGE fixed costs of one queue with the transfers of
            # the other
            for g in range(g_tiles):
                if g % queues != q:
                    continue
                t, cc = divmod(g % n_tiles, n_chunks)
                if g >= bufs:
                    # slot reuse: ScalarE finished reading this buffer
                    # (act g-bufs also implies load g-bufs completed)
                    eng.wait_ge(act_sem, g - bufs + 1)
                eng.dma_start(
                    out=inp[g % bufs][:],
                    in_=x[t * P : (t + 1) * P, cc * col_chunk : (cc + 1) * col_chunk],
                ).then_inc(load_sems[g % n_tiles], 16)
            if q == 0:
                eng.wait_ge(dve_sem, g_rtiles)
                eng.dma_start(out=out[:], in_=sumexp[:]).then_inc(out_sem, 16)
                eng.wait_ge(out_sem, 16)

        @block.sync
        def _(sync):
            load_prog(sync, 0)

        if queues > 1:

            @block.gpsimd
            def _(gpsimd):
                load_prog(gpsimd, 1)

        @block.scalar
        def _(scalar):
            for g in range(g_tiles):
                # exact-max wait on this load slot's sem: engine skew on the
                # 16 SDMA lanes cannot alias it to an incomplete DMA
                scalar.wait_ge(load_sems[g % n_tiles], 16 * (g // n_tiles + 1))
                # In-place exp: the elementwise output is unused (only
                # accum_out matters), and writing back into the input tile
                # keeps every WAW edge semaphore-ordered (act g -> load
                # g+bufs -> act g+bufs).
                scalar.activation(
                    inp[g % bufs][:],
                    inp[g % bufs][:],
                    mybir.ActivationFunctionType.Exp,
                    accum_out=stats[:, g : g + 1],
                ).then_inc(act_sem, 1)

        @block.vector
        def _(vector):
            for t in range(g_rtiles):
                vector.wait_ge(act_sem, n_chunks * (t + 1))
                vector.reduce_sum(
                    sumexp[:, t : t + 1],
                    stats[:, t * n_chunks : (t + 1) * n_chunks],
                    axis=mybir.AxisListType.X,
                ).then_inc(dve_sem, 1)

    return nc


def _run(logits_f32, trace=False, **kwargs):
    """Run the SPMD kernel; returns (sumexp[B] f32, BassKernelResults)."""
    global _NC
    if _NC is None:
        _NC = _build_nc()
    shards = logits_f32.reshape(N_CORES, ROWS_PER_CORE, C)
    in_maps = [{"x": shards[i]} for i in range(N_CORES)]
    res = run_bass_kernel_spmd(_NC, in_maps, list(range(N_CORES)), trace=trace, **kwargs)
    outs = np.stack([res.results[i]["out"] for i in range(N_CORES)])  # [8, 128, 4]
    # out[core][p, t] = sumexp of global row core*512 + t*128 + p
    sumexp = np.transpose(outs, (0, 2, 1)).reshape(B)
    return sumexp, res


def kernel(logits, targets):
    logits = np.ascontiguousarray(np.asarray(logits), dtype=np.float32)
    targets = np.asarray(targets).astype(np.int64)
    assert logits.shape == (B, C)

    sumexp, _ = _run(logits)

    lse = np.log(sumexp.astype(np.float64))
    tgt_logits = logits[np.arange(B), targets].astype(np.float64)
    ce = np.float32(np.mean(lse - tgt_logits))

    # targets.view(B, -1) is [B, 1] -> uniq = 1 per row -> repeated = C - 1
    penalty = np.float32(PENALTY * (C - 1) * B)
    return np.asarray(np.float32(ce) + penalty, dtype=np.float32)
